# revision 18
# baseline (speedup 1.0000x reference)
"""Trainium2 Bass kernel for nn_Attention_9088150798538.

Multi-head causal attention (GQA 16Q/8KV heads, head_dim=128, RoPE) with
in/out projections, B=4, T=2048, d_model=2048, fp32 I/O.

Sharding (8 NeuronCores): core c handles batch b = c//2 and query-head half
hh = c%2 (8 Q heads + 4 KV heads). Out-projection is row-sharded; the two
partial products per batch are summed on the host (with all bias terms
folded into a single host-side vector, since softmax rows sum to 1 the
V-bias passes through attention unchanged).

Device compute is bf16 on the TensorEngine with fp32 PSUM accumulation.
Softmax skips max-subtraction (scores are O(1) by construction).

Engine balance: the PE streams all matmuls; ACT does projections + exp;
VectorE does RoPE muls, reciprocal and the final per-head normalization;
GpSimd owns the softmax-denominator accumulation chain (esum += exp tiles)
and the causal triangle masks. The denominator partition-reduction is one
ones-matmul per (chunk, head), software-pipelined one head behind.

Causal masking is at 128-column granularity: fully-masked column ranges of
diagonal key-tiles are never computed (QK, exp, AV all trimmed); diagonal
scores are packed adjacently so each step is a single contiguous exp.

Scheduling: attention is ACT/DVE-heavy while out-projection is pure PE
work, so out-proj tiles of chunk c-1 are interleaved into the attention
head loop of chunk c (and the deferred half-1 V-projection into chunk 0),
keeping the PE busy while ACT works through the exps. AV matmuls lag the
scores by two steps so exp latency never stalls the PE.
"""
import sys

sys.path.insert(0, "/opt/trn_rl_repo")

import math
import numpy as np
import ml_dtypes

BF16NP = ml_dtypes.bfloat16

D = 2048          # d_model
T = 2048          # sequence length
B = 4             # batch
HD = 128          # head dim
NH = 16           # query heads (global)
NKV = 8           # kv heads (global)
HQ_L = 8          # query heads per core
HKV_L = 4         # kv heads per core
KB = 16           # contraction blocks (D/128)
NCHUNK = 4        # token chunks of 512
SCALE = 1.0 / math.sqrt(HD)
N_WARMUP = 32     # PE warmup matmuls during the initial DMA window
SWAP_MASK = [i ^ 1 for i in range(32)]  # DVE 32-lane pair-swap shuffle

_CACHE = {}


def _build_nc():
    import concourse.bass as bass
    import concourse.mybir as mybir
    import concourse.tile as tile
    from concourse import bacc
    from contextlib import ExitStack

    BF16 = mybir.dt.bfloat16
    F32 = mybir.dt.float32

    nc = bacc.Bacc("TRN2", debug=False, enable_asserts=False,
                   target_bir_lowering=False)

    xT_d = nc.dram_tensor("xT", [D, T], BF16, kind="ExternalInput").ap()
    wqkT_d = nc.dram_tensor("wqkT", [D, 1536], BF16, kind="ExternalInput").ap()
    wvT_d = nc.dram_tensor("wvT", [D, 512], BF16, kind="ExternalInput").ap()
    bqk_d = nc.dram_tensor("bqk", [1536], F32, kind="ExternalInput").ap()
    woT_d = nc.dram_tensor("woT", [1024, D], BF16, kind="ExternalInput").ap()
    cos_d = nc.dram_tensor("cosT", [128, T], BF16, kind="ExternalInput").ap()
    sin_d = nc.dram_tensor("sinT", [128, T], BF16, kind="ExternalInput").ap()
    tri_d = nc.dram_tensor("trib", [128, 128], BF16, kind="ExternalInput").ap()
    y_d = nc.dram_tensor("y", [T, D], F32, kind="ExternalOutput").ap()

    Exp = mybir.ActivationFunctionType.Exp
    Ident = mybir.ActivationFunctionType.Identity

    with tile.TileContext(nc) as tc, ExitStack() as ctx:
        consts = ctx.enter_context(tc.tile_pool(name="consts", bufs=1))
        qkpool = ctx.enter_context(tc.tile_pool(name="qkp", bufs=1))
        vpool = ctx.enter_context(tc.tile_pool(name="vp", bufs=1))
        # xt half-1 and wv outlive phase 1 (deferred half-1 V-projection)
        xpool1 = ctx.enter_context(tc.tile_pool(name="xp1", bufs=1))
        wvpool = ctx.enter_context(tc.tile_pool(name="wvp", bufs=1))

        tri_sb = consts.tile([128, 128], BF16)
        bqk_sb = consts.tile([128, 12], F32)
        ones_kt = consts.tile([128, 128], BF16)

        qkT = qkpool.tile([128, 12, T], BF16)   # [d, ch-block, tok] q0..7 k0..3
        vsb = vpool.tile([128, KB, 512], BF16)  # [tok%128, tok-block, v-ch]
        wv_sb = wvpool.tile([128, KB, 512], BF16)
        xt1 = xpool1.tile([128, KB, 1024], BF16)

        xT_r = xT_d.rearrange("(k p) t -> p k t", p=128)
        wqkT_r = wqkT_d.rearrange("(k p) c -> p k c", p=128)
        wvT_r = wvT_d.rearrange("(k p) c -> p k c", p=128)

        # ---------------- phase 1: projections + RoPE ----------------
        with ExitStack() as p1:
            cspool = p1.enter_context(tc.tile_pool(name="csp", bufs=1))
            xpool0 = p1.enter_context(tc.tile_pool(name="xp0", bufs=1))
            wmpool = p1.enter_context(tc.tile_pool(name="wmp", bufs=5))
            tmppool = p1.enter_context(tc.tile_pool(name="tmpp", bufs=3))
            shpool = p1.enter_context(tc.tile_pool(name="shp", bufs=2))
            t1pool = p1.enter_context(tc.tile_pool(name="t1p", bufs=2))
            projps = p1.enter_context(tc.tile_pool(name="pps", bufs=4, space="PSUM"))
            warmps = p1.enter_context(tc.tile_pool(name="wps", bufs=1, space="PSUM"))

            cos_sb = cspool.tile([128, T], BF16)
            sin_sb = cspool.tile([128, T], BF16)

            def emit_wm_dmas(eng, wm, m):
                for kq in range(4):
                    eng.dma_start(
                        out=wm[:, kq * 4:(kq + 1) * 4, :],
                        in_=wqkT_r[:, kq * 4:(kq + 1) * 4,
                                   m * 128:(m + 1) * 128])

            # PE warmup: keep the array busy (and the HAM clock warm)
            # while the first input DMAs land.
            nc.vector.memset(ones_kt, 1.0)
            wp = warmps.tile([128, 128], F32)
            for _ in range(N_WARMUP):
                nc.tensor.matmul(wp, ones_kt, ones_kt, start=True, stop=True)

            wm0 = None
            for half in range(2):
                toff0 = half * 1024
                if half == 0:
                    xt = xpool0.tile([128, KB, 1024], BF16)
                else:
                    xt = xt1
                if half == 0:
                    # Startup DMA split across both HWDGE queues:
                    #  sync:   wm(m=0), xt n=0 slices
                    #  scalar: wm(m=1), wm(m=2), consts, xt n=1 slices
                    wm0 = wmpool.tile([128, KB, 128], BF16, tag="wm", name="wm")
                    emit_wm_dmas(nc.sync, wm0, 0)
                    for kb in range(8):
                        nc.sync.dma_start(out=xt[:, kb, 0:512],
                                          in_=xT_r[:, kb, 0:512])
                    for kb in range(8, KB):
                        nc.scalar.dma_start(out=xt[:, kb, 0:512],
                                            in_=xT_r[:, kb, 0:512])
                    wm1 = wmpool.tile([128, KB, 128], BF16, tag="wm", name="wm")
                    emit_wm_dmas(nc.scalar, wm1, 1)
                    wm2 = wmpool.tile([128, KB, 128], BF16, tag="wm", name="wm")
                    emit_wm_dmas(nc.scalar, wm2, 2)
                    nc.scalar.dma_start(
                        out=bqk_sb, in_=bqk_d.rearrange("(m p) -> p m", p=128))
                    nc.scalar.dma_start(out=tri_sb, in_=tri_d)
                    nc.scalar.dma_start(out=cos_sb, in_=cos_d)
                    nc.scalar.dma_start(out=sin_sb, in_=sin_d)
                    for kb in range(KB):
                        nc.scalar.dma_start(out=xt[:, kb, 512:1024],
                                            in_=xT_r[:, kb, 512:1024])
                    wms = {0: wm0, 1: wm1, 2: wm2}
                else:
                    for kb in range(KB):
                        nc.sync.dma_start(out=xt[:, kb, :],
                                          in_=xT_r[:, kb, toff0:toff0 + 1024])
                    wms = {}
                # Q and K projections (transposed layout [ch, tok]).
                # half 0 runs m=0..2 over n=0 first so the n=1 token slice
                # (second DMA queue) has time to land.
                if half == 0:
                    mn_order = [(0, 0), (1, 0), (2, 0), (0, 1), (1, 1),
                                (2, 1)] + [(m, n) for m in range(3, 12)
                                           for n in range(2)]
                else:
                    mn_order = [(m, n) for m in range(12) for n in range(2)]
                for m, n in mn_order:
                    wm = wms.get(m)
                    if wm is None:
                        wm = wmpool.tile([128, KB, 128], BF16, tag="wm",
                                         name="wm")
                        emit_wm_dmas(nc.sync, wm, m)
                        wms[m] = wm
                    if True:
                        toff = toff0 + n * 512
                        pp = projps.tile([128, 512], F32)
                        for k in range(KB):
                            nc.tensor.matmul(pp, wm[:, k, :],
                                             xt[:, k, n * 512:(n + 1) * 512],
                                             start=(k == 0), stop=(k == KB - 1))
                        tp = tmppool.tile([128, 512], BF16)
                        nc.scalar.activation(tp, pp, Ident,
                                             bias=bqk_sb[:, m:m + 1])
                        # RoPE: pair-swap via DVE shuffle; rotation sign is
                        # folded into the host sin table
                        sh = shpool.tile([128, 512], BF16)
                        nc.vector.stream_shuffle(sh, tp, mask=SWAP_MASK)
                        t1 = t1pool.tile([128, 512], BF16)
                        nc.vector.tensor_mul(t1, tp, cos_sb[:, toff:toff + 512])
                        nc.vector.tensor_mul(sh, sh, sin_sb[:, toff:toff + 512])
                        nc.vector.tensor_add(qkT[:, m, toff:toff + 512], t1, sh)
                # V projection, half 0 only (half 1 deferred into attention)
                if half == 0:
                    for kq in range(4):
                        nc.sync.dma_start(out=wv_sb[:, kq * 4:(kq + 1) * 4, :],
                                          in_=wvT_r[:, kq * 4:(kq + 1) * 4, :])
                    for tbl in range(8):
                        pp = projps.tile([128, 512], F32)
                        for k in range(KB):
                            nc.tensor.matmul(pp,
                                             xt[:, k, tbl * 128:(tbl + 1) * 128],
                                             wv_sb[:, k, :],
                                             start=(k == 0), stop=(k == KB - 1))
                        nc.scalar.copy(vsb[:, tbl, :], pp)

        # -------- phase 2: attention + interleaved out-projection --------
        with ExitStack() as p2:
            wopool = p2.enter_context(tc.tile_pool(name="wop", bufs=1))
            otpool = p2.enter_context(tc.tile_pool(name="otp", bufs=1))
            youtpool = p2.enter_context(tc.tile_pool(name="yop", bufs=4))
            wo_sb = wopool.tile([128, 8, D], BF16)
            woT_r = woT_d.rearrange("(g p) o -> p g o", p=128)
            for g in range(8):
                nc.sync.dma_start(out=wo_sb[:, g, :], in_=woT_r[:, g, :])
            otT = otpool.tile([128, 8, T], BF16)  # [d, head, tok]

            outproj_q = []   # (tb, oc) tiles ready to emit
            vdef_q = list(range(8))  # deferred half-1 V-proj token blocks

            with ExitStack() as pa:
                epool = pa.enter_context(tc.tile_pool(name="ep", bufs=4))
                espool = pa.enter_context(tc.tile_pool(name="esp", bufs=2))
                # esum+esumb per head, pending + current in flight
                rbpool = pa.enter_context(tc.tile_pool(name="rbp", bufs=2))
                stps = pa.enter_context(tc.tile_pool(name="stps", bufs=2, space="PSUM"))
                otps = pa.enter_context(tc.tile_pool(name="otps", bufs=2, space="PSUM"))
                dps = pa.enter_context(tc.tile_pool(name="dps", bufs=1, space="PSUM"))
                auxps = pa.enter_context(tc.tile_pool(name="auxps", bufs=1, space="PSUM"))

                def emit_outproj_tile(tb, oc, copy_eng):
                    tsl = slice(tb * 128, (tb + 1) * 128)
                    yp = auxps.tile([128, 512], F32, tag="aux", name="aux")
                    for ii in range(HQ_L):
                        nc.tensor.matmul(
                            yp, otT[:, ii, tsl],
                            wo_sb[:, ii, oc * 512:(oc + 1) * 512],
                            start=(ii == 0), stop=(ii == HQ_L - 1))
                    yo = youtpool.tile([128, 512], F32)
                    if copy_eng == "scalar":
                        nc.scalar.copy(yo, yp)
                    else:
                        nc.vector.tensor_copy(yo, yp)
                    nc.sync.dma_start(
                        out=y_d[tsl, oc * 512:(oc + 1) * 512], in_=yo)

                def emit_vdef_tile(tbl):
                    pp = auxps.tile([128, 512], F32, tag="aux", name="aux")
                    for k in range(KB):
                        nc.tensor.matmul(pp,
                                         xt1[:, k, tbl * 128:(tbl + 1) * 128],
                                         wv_sb[:, k, :],
                                         start=(k == 0), stop=(k == KB - 1))
                    nc.scalar.copy(vsb[:, 8 + tbl, :], pp)

                filler_count = [0]

                def emit_filler():
                    # PE-only work injected into ACT-bound attention stretches
                    if vdef_q:
                        emit_vdef_tile(vdef_q.pop(0))
                    elif outproj_q:
                        tb, oc = outproj_q.pop(0)
                        eng = "scalar" if filler_count[0] % 2 else "vector"
                        filler_count[0] += 1
                        emit_outproj_tile(tb, oc, eng)

                pending = [None]  # (esum, otp, head, qsl) awaiting denominator

                def flush_pending():
                    esum_p, esumb_p, otp_p, i_p, qsl_p = pending[0]
                    pending[0] = None
                    nc.vector.tensor_add(esum_p, esum_p, esumb_p)
                    dp = dps.tile([128, 512], F32)
                    nc.tensor.matmul(dp, ones_kt, esum_p, start=True, stop=True)
                    rb = rbpool.tile([128, 512], F32)
                    nc.vector.reciprocal_approx_fast(rb, dp)
                    nc.vector.tensor_mul(otT[:, i_p, qsl_p], otp_p, rb)

                for c in range(NCHUNK):
                    qsl = slice(c * 512, (c + 1) * 512)
                    nkt = 4 * c + 4
                    nst = nkt // 2
                    # diagonal (masked/trimmed) steps first so the head's
                    # tail exp is always a full, mask-free one
                    step_order = ([2 * c, 2 * c + 1] + list(range(2 * c))
                                  if c > 0 else list(range(nst)))
                    flush_idx = 2 if nst > 2 else 1
                    for i in range(HQ_L):
                        # head 0 of a chunk: delay fillers so the freshly
                        # flushed otT of head (c-1,7) has time to land
                        if nst > 2:
                            fill_idxs = {3, nst // 2 + 2} if i == 0 else \
                                        {1, nst // 2 + 1}
                        else:
                            fill_idxs = {1}
                        kv = i // 2
                        kvsl = slice(kv * 128, (kv + 1) * 128)
                        otp = otps.tile([128, 512], F32)
                        esum = espool.tile([128, 512], BF16, tag="esum", bufs=2)
                        esumb = espool.tile([128, 512], BF16, tag="esumb", bufs=2)

                        def emit_av(item, last):
                            es, c00, c01, kt0, kt1, first = item
                            nc.tensor.matmul(
                                otp[:, c00:512], vsb[:, kt0, kvsl],
                                es[:, c00:512], start=first, stop=False)
                            nc.tensor.matmul(
                                otp[:, c01:512], vsb[:, kt1, kvsl],
                                es[:, 512:1024 - c01], start=False, stop=last)

                        avq = []
                        for idx, s in enumerate(step_order):
                            kt0, kt1 = 2 * s, 2 * s + 1
                            j0 = kt0 - 4 * c   # >= 0 -> diagonal key-tile
                            j1 = kt1 - 4 * c
                            c00 = 128 * j0 if j0 > 0 else 0
                            c01 = 128 * j1 if j1 > 0 else 0
                            stp = stps.tile([128, 1024], F32)
                            nc.tensor.matmul(
                                stp[:, c00:512],
                                qkT[:, 8 + kv, kt0 * 128:(kt0 + 1) * 128],
                                qkT[:, i, c * 512 + c00:(c + 1) * 512],
                                start=True, stop=True)
                            # kt1 scores packed right after kt0's so the
                            # step is one contiguous exp
                            nc.tensor.matmul(
                                stp[:, 512:1024 - c01],
                                qkT[:, 8 + kv, kt1 * 128:(kt1 + 1) * 128],
                                qkT[:, i, c * 512 + c01:(c + 1) * 512],
                                start=True, stop=True)
                            if idx == flush_idx and pending[0] is not None:
                                flush_pending()
                            if idx in fill_idxs:
                                emit_filler()
                            if len(avq) >= 2:
                                emit_av(avq.pop(0), last=False)
                            es = epool.tile([128, 1024], BF16)
                            nc.scalar.activation(es[:, c00:1024 - c01],
                                                 stp[:, c00:1024 - c01],
                                                 Exp, scale=SCALE)
                            if j0 >= 0:
                                nc.vector.tensor_mul(
                                    es[:, c00:c00 + 128],
                                    es[:, c00:c00 + 128], tri_sb)
                                nc.vector.tensor_mul(
                                    es[:, 512:640], es[:, 512:640], tri_sb)
                            if idx == 0:
                                nc.vector.tensor_copy(esum, es[:, 0:512])
                                if c01:
                                    nc.gpsimd.memset(esumb[:, 0:c01], 0.0)
                                nc.gpsimd.tensor_copy(
                                    esumb[:, c01:512], es[:, 512:1024 - c01])
                            else:
                                nc.vector.tensor_add(
                                    esum[:, c00:512], esum[:, c00:512],
                                    es[:, c00:512])
                                nc.gpsimd.tensor_add(
                                    esumb[:, c01:512], esumb[:, c01:512],
                                    es[:, 512:1024 - c01])
                            avq.append((es, c00, c01, kt0, kt1, idx == 0))
                        emit_av(avq.pop(0), last=False)
                        emit_av(avq.pop(0), last=True)
                        pending[0] = (esum, esumb, otp, i, qsl)
                    # chunk c done: its out-proj tiles become legal once
                    # head (c,7)'s denominator flushes (next head's idx 1)
                    if c > 0:
                        for tb in range(4 * (c - 1), 4 * c):
                            for oc in range(4):
                                outproj_q.append((tb, oc))
                flush_pending()

            # drain: remaining out-proj tiles (incl. all of chunk 3)
            outproj_q.extend(
                (tb, oc) for tb in range(12, 16) for oc in range(4))
            with ExitStack() as po:
                yps = po.enter_context(tc.tile_pool(name="yps", bufs=4, space="PSUM"))
                for di, (tb, oc) in enumerate(outproj_q):
                    tsl = slice(tb * 128, (tb + 1) * 128)
                    yp = yps.tile([128, 512], F32)
                    for ii in range(HQ_L):
                        nc.tensor.matmul(
                            yp, otT[:, ii, tsl],
                            wo_sb[:, ii, oc * 512:(oc + 1) * 512],
                            start=(ii == 0), stop=(ii == HQ_L - 1))
                    yo = youtpool.tile([128, 512], F32)
                    if di % 2:
                        nc.scalar.copy(yo, yp)
                    else:
                        nc.vector.tensor_copy(yo, yp)
                    deng = nc.scalar if di % 2 else nc.sync
                    deng.dma_start(
                        out=y_d[tsl, oc * 512:(oc + 1) * 512], in_=yo)

    nc.compile()
    return nc


def _get_nc():
    if "nc" not in _CACHE:
        _CACHE["nc"] = _build_nc()
    return _CACHE["nc"]


def _host_tables():
    if "tables" in _CACHE:
        return _CACHE["tables"]
    inv = 1.0 / (10000.0 ** (np.arange(0, HD, 2, dtype=np.float64) / HD))
    freqs = np.arange(T, dtype=np.float64)[:, None] * inv[None, :]  # [T, 64]
    cosT = np.repeat(np.cos(freqs).T, 2, axis=0).astype(BF16NP)  # [128, T]
    sinT = np.repeat(np.sin(freqs).T, 2, axis=0)
    sinT[0::2] *= -1.0   # rotation sign folded into the table (shuffle RoPE)
    sinT = sinT.astype(BF16NP)
    p = np.arange(128)[:, None]
    f = np.arange(128)[None, :]
    trib = (f >= p).astype(BF16NP)
    _CACHE["tables"] = (cosT, sinT, trib)
    return _CACHE["tables"]


def kernel(x, Wq, bq, Wkv, bkv, Wo, bo):
    from concourse import bass_utils

    nc = _get_nc()
    cosT, sinT, trib = _host_tables()

    x = np.asarray(x, np.float32)
    Wq = np.asarray(Wq, np.float32)
    bq = np.asarray(bq, np.float32)
    Wkv = np.asarray(Wkv, np.float32)
    bkv = np.asarray(bkv, np.float32)
    Wo = np.asarray(Wo, np.float32)
    bo = np.asarray(bo, np.float32)

    in_maps = []
    bias_vecs = np.zeros((2, D), np.float32)
    percore = {}
    for hh in range(2):
        wq_h = Wq[hh * 1024:(hh + 1) * 1024, :]
        wk_h = Wkv[hh * 512:(hh + 1) * 512, :]
        wv_h = Wkv[1024 + hh * 512:1024 + (hh + 1) * 512, :]
        wqkT = np.ascontiguousarray(
            np.concatenate([wq_h, wk_h], axis=0).T).astype(BF16NP)
        wvT = np.ascontiguousarray(wv_h.T).astype(BF16NP)
        bqk = np.concatenate([bq[hh * 1024:(hh + 1) * 1024],
                              bkv[hh * 512:(hh + 1) * 512]]).astype(np.float32)
        woT = np.ascontiguousarray(
            Wo[:, hh * 1024:(hh + 1) * 1024].T).astype(BF16NP)
        percore[hh] = (wqkT, wvT, bqk, woT)
        bv_h = bkv[1024 + hh * 512:1024 + (hh + 1) * 512]
        bv_expand = np.concatenate(
            [bv_h[(i // 2) * 128:(i // 2 + 1) * 128] for i in range(HQ_L)])
        bias_vecs[hh] = bv_expand @ Wo[:, hh * 1024:(hh + 1) * 1024].T

    for c in range(8):
        b, hh = divmod(c, 2)
        xT = np.ascontiguousarray(x[b].T).astype(BF16NP)
        wqkT, wvT, bqk, woT = percore[hh]
        in_maps.append({
            "xT": xT, "wqkT": wqkT, "wvT": wvT, "bqk": bqk, "woT": woT,
            "cosT": cosT, "sinT": sinT, "trib": trib,
        })

    res = bass_utils.run_bass_kernel_spmd(nc, in_maps, core_ids=list(range(8)),
                                          trace=False)
    bias_vec = (bo + bias_vecs[0] + bias_vecs[1]).astype(np.float32)
    out = np.empty((B, T, D), np.float32)
    for b in range(B):
        out[b] = res.results[2 * b]["y"] + res.results[2 * b + 1]["y"] + bias_vec
    return out
